# revision 10
# baseline (speedup 1.0000x reference)
"""Multi-head attention Trainium2 kernel (8 NeuronCores, SPMD).

Sharding: core c handles batch b = c//4 and the 4 heads [4*(c%4), 4*(c%4)+4).
Attention is fully independent per (batch, head); the output projection is
computed as per-core partial sums over the core's 256 ctx columns and reduced
on the host (plus bias).

Per-core device program (all matmuls contract along the partition dim):
  xT    = transpose(x)                      [e, sq]   via PE transposes
  qT/kT = W @ xT + b                        [dq, sq]  (head dim on partitions)
  v     = x @ WvT + bv                      [t, dv]   (natural layout)
  per head, per 1024-column chunk of sq:
    sT   = kT_h^T-contract-> [t, sq] tiles  (scores transposed)
    A    = exp(sT/8)  (ACT, PSUM->SBUF)
    ctxT/rowsum via [V | 1] ones-column matmul, contract over t
    normalize with PE-broadcast reciprocal row; write A^T strips to HBM
  outT  = WoT @ ctxT                        [do, sq]  partial, host-reduced

The attention tensor is produced transposed ([h, t, sq] per core); kernel()
returns a numpy transposed view so no host transpose copy is needed.

Env knobs:
  ATTN_MM_DTYPE = f32r (default) | f32   -- matmul operand dtype
"""

import os

import numpy as np

EMBED = 1024
NHEADS = 16
DH = 64
BSZ = 2
SEQ = 2048
NCORES = 8
HPC = 4          # heads per core
DQ = HPC * DH    # 256 projection columns per core

_MM_MODE = os.environ.get("ATTN_MM_DTYPE", "f32r")

_state = {}


def _build_nc():
    import concourse.bacc as bacc
    import concourse.mybir as mybir
    from concourse import masks
    from concourse.tile import TileContext

    F32 = mybir.dt.float32
    MMDT = mybir.dt.float32r if _MM_MODE == "f32r" else F32
    AF = mybir.ActivationFunctionType

    nc = bacc.Bacc(trn_type="TRN2", target_bir_lowering=False)

    X = nc.declare_dram_parameter("x", [SEQ, EMBED], F32, isOutput=False)
    WQ = nc.declare_dram_parameter("wq", [DQ, EMBED], F32, isOutput=False)
    BQ = nc.declare_dram_parameter("bq", [DQ], F32, isOutput=False)
    WK = nc.declare_dram_parameter("wk", [DQ, EMBED], F32, isOutput=False)
    BK = nc.declare_dram_parameter("bk", [DQ], F32, isOutput=False)
    WV = nc.declare_dram_parameter("wv", [DQ, EMBED], F32, isOutput=False)
    BV = nc.declare_dram_parameter("bv", [DQ], F32, isOutput=False)
    WO = nc.declare_dram_parameter("wo", [EMBED, DQ], F32, isOutput=False)
    ATT = nc.declare_dram_parameter("attnT", [HPC, SEQ, SEQ], F32, isOutput=True)
    OUTT = nc.declare_dram_parameter("outT", [EMBED, SEQ], F32, isOutput=True)

    with TileContext(nc) as tc:
        with (
            tc.tile_pool(name="const", bufs=1) as cp,
            tc.tile_pool(name="persist", bufs=1) as pp,
        ):
            ident = cp.tile([128, 128], F32, tag="ident")
            masks.make_identity(nc, ident[:])
            ones1 = cp.tile([1, 128], F32, tag="ones1")
            nc.vector.memset(ones1[:], 1.0)
            bq_t = cp.tile([128, 2], F32, tag="bqt")
            nc.sync.dma_start(out=bq_t[:], in_=BQ.ap().rearrange("(j p) -> p j", p=128))
            bk_t = cp.tile([128, 2], F32, tag="bkt")
            nc.sync.dma_start(out=bk_t[:], in_=BK.ap().rearrange("(j p) -> p j", p=128))
            bv_row = cp.tile([1, DQ], F32, tag="bvr")
            nc.sync.dma_start(out=bv_row[:], in_=BV.ap().unsqueeze(0))
            warm = cp.tile([1, 16], F32, tag="warm")

            qT = [pp.tile([128, SEQ], MMDT, tag=f"qT{i}", name=f"qT{i}") for i in range(2)]
            kT = [pp.tile([128, SEQ], MMDT, tag=f"kT{i}", name=f"kT{i}") for i in range(2)]
            vsb = pp.tile([128, 16 * HPC * 65], MMDT, tag="vsb")  # [t-tile][head][64+ones]
            ctxT = [pp.tile([128, SEQ], MMDT, tag=f"ctxT{i}", name=f"ctxT{i}") for i in range(2)]
            woT = [pp.tile([128, 1024], MMDT, tag=f"woT{i}", name=f"woT{i}") for i in range(2)]

            # ---------- phase 0: transposes; phase 1: projections ----------
            with (
                tc.tile_pool(name="ph1sb", bufs=1) as xp,
                tc.tile_pool(name="loads", bufs=3) as lp,
                tc.tile_pool(name="pstr", bufs=2, space="PSUM") as pstr,
                tc.tile_pool(name="psproj", bufs=4, space="PSUM") as pspr,
            ):
                xT = xp.tile([128, 8 * SEQ], MMDT, tag="xT")
                wqT = xp.tile([128, 8 * DQ], MMDT, tag="wqT")
                wkT = xp.tile([128, 8 * DQ], MMDT, tag="wkT")
                wvT = xp.tile([128, 8 * DQ], MMDT, tag="wvT")

                # PE warmup past identity creation + ACT exp table preload
                ptw = pstr.tile([128, 1024], F32, tag="ptr")
                nc.tensor.transpose(ptw[:, 0:128], ident[:], ident[:])
                nc.scalar.activation(warm[:], ptw[0:1, 0:16], AF.Exp)

                # Wo -> woT  (woT[j][p, do] = Wo[do, 128j + p]); first so the
                # two concurrent psum tiles can come from the "ptr" slots
                pws = [pstr.tile([128, 1024], F32, tag="ptr", name=f"pwo{j}") for j in range(2)]
                for dt_ in range(8):
                    wl = lp.tile([128, DQ], F32, tag="xload")
                    nc.sync.dma_start(out=wl[:], in_=WO.ap()[128 * dt_:128 * (dt_ + 1), :])
                    for j in range(2):
                        nc.tensor.transpose(
                            pws[j][:, 128 * dt_:128 * (dt_ + 1)],
                            wl[:, 128 * j:128 * (j + 1)],
                            ident[:],
                        )
                for j in range(2):
                    nc.scalar.copy(out=woT[j][:], in_=pws[j][:])

                # x -> xT  (xT[p, SEQ*j + s] = x[s, 128j + p])
                for st in range(16):
                    xl = lp.tile([128, EMBED], F32, tag="xload")
                    nc.sync.dma_start(out=xl[:], in_=X.ap()[128 * st:128 * (st + 1), :])
                    pt = pstr.tile([128, 1024], F32, tag="ptr")
                    for j in range(8):
                        nc.tensor.transpose(
                            pt[:, 128 * j:128 * (j + 1)],
                            xl[:, 128 * j:128 * (j + 1)],
                            ident[:],
                        )
                    dest = xT[:].rearrange("p (j s) -> p j s", j=8)[:, :, 128 * st:128 * (st + 1)]
                    nc.scalar.copy(out=dest, in_=pt[:])

                # Wq/Wk/Wv -> wT  (wT[p, 256j + m] = W[m, 128j + p])
                for W, wT in ((WQ, wqT), (WK, wkT), (WV, wvT)):
                    wls = []
                    for mi in range(2):
                        wl = lp.tile([128, EMBED], F32, tag="xload")
                        nc.sync.dma_start(out=wl[:], in_=W.ap()[128 * mi:128 * (mi + 1), :])
                        wls.append(wl)
                    for g in range(2):
                        pw = pstr.tile([128, 1024], F32, tag="ptr")
                        for jj in range(4):
                            j = 4 * g + jj
                            for mi in range(2):
                                nc.tensor.transpose(
                                    pw[:, 256 * jj + 128 * mi:256 * jj + 128 * (mi + 1)],
                                    wls[mi][:, 128 * j:128 * (j + 1)],
                                    ident[:],
                                )
                        nc.scalar.copy(out=wT[:, 1024 * g:1024 * (g + 1)], in_=pw[:])

                # qT / kT projections (contract e over 8 j-tiles)
                for wT, qk, bt in ((wqT, qT, bq_t), (wkT, kT, bk_t)):
                    for mi in range(2):
                        for n in range(4):
                            pq = pspr.tile([128, 512], F32, tag="pproj")
                            for j in range(8):
                                nc.tensor.matmul(
                                    pq[:],
                                    wT[:, 256 * j + 128 * mi:256 * j + 128 * (mi + 1)],
                                    xT[:, SEQ * j + 512 * n:SEQ * j + 512 * (n + 1)],
                                    start=(j == 0),
                                    stop=(j == 7),
                                )
                            nc.scalar.activation(
                                qk[mi][:, 512 * n:512 * (n + 1)],
                                pq[:],
                                AF.Identity,
                                bias=bt[:, mi:mi + 1],
                                scale=1.0,
                            )

                # v projection into [V | 1] layout (ones cols via ACT casts)
                ones4 = cp.tile([128, 4], F32, tag="ones4")
                nc.vector.memset(ones4[:], 1.0)
                vview = vsb[:].rearrange("p (t h e) -> p t h e", t=16, h=HPC)
                for tt in range(16):
                    nc.scalar.copy(out=vview[:, tt, :, 64:65], in_=ones4[:].unsqueeze(-1))
                    pv = pspr.tile([128, DQ], F32, tag="pproj")
                    for j in range(8):
                        nc.tensor.matmul(
                            pv[:],
                            xT[:, SEQ * j + 128 * tt:SEQ * j + 128 * (tt + 1)],
                            wvT[:, 256 * j:256 * (j + 1)],
                            start=(j == 0),
                            stop=False,
                        )
                    nc.tensor.matmul(pv[:], ones1[:], bv_row[:], start=False, stop=True)
                    nc.scalar.copy(
                        out=vview[:, tt, :, 0:64],
                        in_=pv[:].rearrange("p (h d) -> p h d", h=HPC),
                    )

            # ---------- phase 2: attention ----------
            with (
                tc.tile_pool(name="strips", bufs=3) as sp,
                tc.tile_pool(name="ostrips", bufs=2) as op2,
                tc.tile_pool(name="rp", bufs=2) as rp,
                tc.tile_pool(name="pss", bufs=2, space="PSUM") as pss,
                tc.tile_pool(name="psc", bufs=1, space="PSUM") as psc,
                tc.tile_pool(name="psb", bufs=1, space="PSUM") as psb,
            ):
                for h in range(HPC):
                    mi, po = h // 2, 64 * (h % 2)
                    att_h = ATT.ap()[h].rearrange("(t p) s -> p t s", p=128)
                    for ch in range(2):
                        sq0 = 1024 * ch
                        halves = [sp.tile([128, 8 * 1024], MMDT, tag="strip", name=f"strip{h}_{ch}_{i}") for i in range(2)]
                        pc = psc.tile([65, 1024], F32, tag="ctx")
                        for tt in range(16):
                            hs, tl = tt // 8, tt % 8
                            ps = pss.tile([128, 1024], F32, tag="score")
                            for n in range(2):
                                nc.tensor.matmul(
                                    ps[:, 512 * n:512 * (n + 1)],
                                    kT[mi][po:po + 64, 128 * tt:128 * (tt + 1)],
                                    qT[mi][po:po + 64, sq0 + 512 * n:sq0 + 512 * (n + 1)],
                                    start=True,
                                    stop=True,
                                )
                            nc.scalar.activation(
                                halves[hs][:, 1024 * tl:1024 * (tl + 1)],
                                ps[:],
                                AF.Exp,
                                scale=0.125,
                            )
                            for n in range(2):
                                nc.tensor.matmul(
                                    pc[:, 512 * n:512 * (n + 1)],
                                    vview[:, tt, h, :],
                                    halves[hs][:, 1024 * tl + 512 * n:1024 * tl + 512 * (n + 1)],
                                    start=(tt == 0),
                                    stop=(tt == 15),
                                )
                        rc = rp.tile([1, 1024], F32, tag="recip")
                        nc.vector.reciprocal(rc[:], pc[64:65, :])
                        pb = psb.tile([128, 1024], F32, tag="bcast")
                        for n in range(2):
                            nc.tensor.matmul(
                                pb[:, 512 * n:512 * (n + 1)],
                                ones1[:],
                                rc[0:1, 512 * n:512 * (n + 1)],
                                start=True,
                                stop=True,
                            )
                        ctmp = rp.tile([64, 1024], F32, tag="ctmp", name=f"ctmp{h}_{ch}")
                        nc.scalar.copy(out=ctmp[:], in_=pc[0:64, :])
                        nc.vector.tensor_mul(
                            ctxT[mi][po:po + 64, sq0:sq0 + 1024], ctmp[:], pb[0:64, :]
                        )
                        # normalize A^T into f32 out-strips (2 t-tiles each) + DMA
                        for e in range(8):
                            hs, t0 = e // 4, (e % 4) * 2
                            ostrip = op2.tile([128, 2048], F32, tag="ostrip",
                                              name=f"ostrip{h}_{ch}_{e}")
                            for u in range(2):
                                nc.vector.tensor_mul(
                                    ostrip[:, 1024 * u:1024 * (u + 1)],
                                    halves[hs][:, 1024 * (t0 + u):1024 * (t0 + u + 1)].bitcast(F32),
                                    pb[:],
                                )
                            nc.sync.dma_start(
                                out=att_h[:, 8 * hs + t0:8 * hs + t0 + 2, sq0:sq0 + 1024],
                                in_=ostrip[:].rearrange("p (t s) -> p t s", t=2),
                            )

            # ---------- phase 3: output projection (partial) ----------
            with (
                tc.tile_pool(name="osb", bufs=2) as op,
                tc.tile_pool(name="pso", bufs=4, space="PSUM") as pso,
            ):
                for dt_ in range(8):
                    osb = op.tile([128, SEQ], F32, tag="outT")
                    for n in range(4):
                        po_ = pso.tile([128, 512], F32, tag="po")
                        for it in range(2):
                            nc.tensor.matmul(
                                po_[:],
                                woT[it][:, 128 * dt_:128 * (dt_ + 1)],
                                ctxT[it][:, 512 * n:512 * (n + 1)],
                                start=(it == 0),
                                stop=(it == 1),
                            )
                        nc.scalar.copy(out=osb[:, 512 * n:512 * (n + 1)], in_=po_[:])
                    nc.sync.dma_start(
                        out=OUTT.ap()[128 * dt_:128 * (dt_ + 1), :], in_=osb[:]
                    )

    nc.finalize()
    return nc


def _get_nc():
    if "nc" not in _state:
        _state["nc"] = _build_nc()
    return _state["nc"]


def _shard_inputs(x, Wq, bq, Wk, bk, Wv, bv, Wo, bo):
    f = lambda a: np.ascontiguousarray(np.asarray(a, dtype=np.float32))
    x, Wq, bq, Wk, bk, Wv, bv, Wo, bo = map(f, (x, Wq, bq, Wk, bk, Wv, bv, Wo, bo))
    in_maps = []
    for c in range(NCORES):
        b, hb = c // 4, c % 4
        sl = slice(DQ * hb, DQ * (hb + 1))
        in_maps.append({
            "x": x[b],
            "wq": np.ascontiguousarray(Wq[sl]), "bq": np.ascontiguousarray(bq[sl]),
            "wk": np.ascontiguousarray(Wk[sl]), "bk": np.ascontiguousarray(bk[sl]),
            "wv": np.ascontiguousarray(Wv[sl]), "bv": np.ascontiguousarray(bv[sl]),
            "wo": np.ascontiguousarray(Wo[:, sl]),
        })
    return in_maps, bo


def kernel(x, Wq, bq, Wk, bk, Wv, bv, Wo, bo):
    from concourse.bass_utils import run_bass_kernel_spmd

    nc = _get_nc()
    in_maps, bo_np = _shard_inputs(x, Wq, bq, Wk, bk, Wv, bv, Wo, bo)
    res = run_bass_kernel_spmd(nc, in_maps, core_ids=list(range(NCORES)))
    _state["last"] = res

    attnT = np.empty((BSZ, NHEADS, SEQ, SEQ), np.float32)
    out = np.zeros((BSZ, SEQ, EMBED), np.float32)
    for c in range(NCORES):
        b, hb = c // 4, c % 4
        r = res.results[c]
        attnT[b, HPC * hb:HPC * (hb + 1)] = r["attnT"]
        out[b] += r["outT"].T
    out += bo_np
    return out, attnT.swapaxes(2, 3)


# revision 15
# speedup vs baseline: 1.0674x; 1.0674x over previous
"""Multi-head attention Trainium2 kernel (8 NeuronCores, SPMD).

Sharding: core c handles batch b = c//4 and the 4 heads [4*(c%4), 4*(c%4)+4).
Attention is fully independent per (batch, head); the output projection is
computed as per-core partial sums over the core's 256 ctx columns and reduced
on the host (plus bias).

Host pre-transposes x and the weight blocks so the device loads operands
directly in contraction-on-partition layout (no on-chip transposes).

Per-core device program (matmuls contract along the partition dim):
  qT/kT = W @ xT + b                        [dq, sq]  (head dim on partitions)
  v     = x @ WvT + bv                      [t, dv]   (natural layout)
  per head, per 1024-column chunk of sq:
    sT   = scores transposed [t, sq] tiles (lhsT = kT slice, rhs = qT slice)
    A    = exp(sT/8)  (ACT, PSUM->SBUF, rounded to matmul dtype)
    ctxT + row-sums via [V | 1] ones-column matmul, contract over t
    reciprocal via PE row<->column transposes (128-lane DVE recip)
    normalize with PE-broadcast reciprocal row; DMA A^T strips to HBM
  outT  = WoT @ ctxT                        [do, sq]  partial, host-reduced

The attention tensor is produced transposed ([h, t, sq] per core); kernel()
returns a numpy transposed view so no host transpose copy is needed.

Env knobs:
  ATTN_MM_DTYPE = f32r (default) | f32   -- matmul operand dtype
"""

import os

import numpy as np

EMBED = 1024
NHEADS = 16
DH = 64
BSZ = 2
SEQ = 2048
NCORES = 8
HPC = 4          # heads per core
DQ = HPC * DH    # 256 projection columns per core

_MM_MODE = os.environ.get("ATTN_MM_DTYPE", "f32r")

_state = {}


def _build_nc():
    import concourse.bacc as bacc
    import concourse.mybir as mybir
    from concourse import masks
    from concourse.tile import TileContext

    F32 = mybir.dt.float32
    MMDT = mybir.dt.float32r if _MM_MODE == "f32r" else F32
    AF = mybir.ActivationFunctionType

    nc = bacc.Bacc(trn_type="TRN2", target_bir_lowering=False)

    XT = nc.declare_dram_parameter("xt", [EMBED, SEQ], F32, isOutput=False)
    WQT = nc.declare_dram_parameter("wqt", [EMBED, DQ], F32, isOutput=False)
    BQ = nc.declare_dram_parameter("bq", [DQ], F32, isOutput=False)
    WKT = nc.declare_dram_parameter("wkt", [EMBED, DQ], F32, isOutput=False)
    BK = nc.declare_dram_parameter("bk", [DQ], F32, isOutput=False)
    WVT = nc.declare_dram_parameter("wvt", [EMBED, DQ], F32, isOutput=False)
    BV = nc.declare_dram_parameter("bv", [DQ], F32, isOutput=False)
    WOT = nc.declare_dram_parameter("wot", [DQ, EMBED], F32, isOutput=False)
    ATT = nc.declare_dram_parameter("attnT", [HPC, SEQ, SEQ], F32, isOutput=True)
    OUTT = nc.declare_dram_parameter("outT", [EMBED, SEQ], F32, isOutput=True)

    with TileContext(nc) as tc:
        with (
            tc.tile_pool(name="const", bufs=1) as cp,
            tc.tile_pool(name="persist", bufs=1) as pp,
        ):
            ident = cp.tile([128, 128], F32, tag="ident")
            masks.make_identity(nc, ident[:])
            ones1 = cp.tile([1, 128], F32, tag="ones1")
            nc.vector.memset(ones1[:], 1.0)
            ones4 = cp.tile([128, 4], F32, tag="ones4")
            nc.vector.memset(ones4[:], 1.0)
            bq_t = cp.tile([128, 2], F32, tag="bqt")
            nc.sync.dma_start(out=bq_t[:], in_=BQ.ap().rearrange("(j p) -> p j", p=128))
            bk_t = cp.tile([128, 2], F32, tag="bkt")
            nc.sync.dma_start(out=bk_t[:], in_=BK.ap().rearrange("(j p) -> p j", p=128))
            bv_row = cp.tile([1, DQ], F32, tag="bvr")
            nc.sync.dma_start(out=bv_row[:], in_=BV.ap().unsqueeze(0))
            warm = cp.tile([1, 2], F32, tag="warm")
            nc.scalar.activation(warm[:], bq_t[0:1, 0:2], AF.Exp)

            qT = [pp.tile([128, SEQ], MMDT, tag=f"qT{i}", name=f"qT{i}") for i in range(2)]
            kT = [pp.tile([128, SEQ], MMDT, tag=f"kT{i}", name=f"kT{i}") for i in range(2)]
            vsb = pp.tile([128, 16 * HPC * 65], MMDT, tag="vsb")  # [t-tile][head][64+ones]
            ctxT = [pp.tile([128, SEQ], MMDT, tag=f"ctxT{i}", name=f"ctxT{i}") for i in range(2)]
            woT = [pp.tile([128, 1024], MMDT, tag=f"woT{i}", name=f"woT{i}") for i in range(2)]

            # woT loads (cast-DMA f32 -> mm dtype on gpsimd SWDGE)
            for j in range(2):
                nc.gpsimd.dma_start(out=woT[j][:], in_=WOT.ap()[128 * j:128 * (j + 1), :])

            # ---------- phase 1: projections ----------
            with (
                tc.tile_pool(name="ph1sb", bufs=1) as xp,
                tc.tile_pool(name="psproj", bufs=4, space="PSUM") as pspr,
            ):
                xT = xp.tile([128, 8 * SEQ], MMDT, tag="xT")
                wqT = xp.tile([128, 8 * DQ], MMDT, tag="wqT")
                wkT = xp.tile([128, 8 * DQ], MMDT, tag="wkT")
                wvT = xp.tile([128, 8 * DQ], MMDT, tag="wvT")

                nc.gpsimd.dma_start(
                    out=xT[:], in_=XT.ap().rearrange("(j p) s -> p j s", p=128)
                )
                for W, wT in ((WQT, wqT), (WKT, wkT), (WVT, wvT)):
                    nc.gpsimd.dma_start(
                        out=wT[:], in_=W.ap().rearrange("(j p) m -> p j m", p=128)
                    )

                # qT / kT: lhsT (weight slice) reused across the 4 n-chunks
                for wT, qk, bt in ((wqT, qT, bq_t), (wkT, kT, bk_t)):
                    for mi in range(2):
                        pq = [pspr.tile([128, 512], F32, tag="pproj", name=f"pq{mi}_{n}")
                              for n in range(4)]
                        for j in range(8):
                            for n in range(4):
                                nc.tensor.matmul(
                                    pq[n][:],
                                    wT[:, 256 * j + 128 * mi:256 * j + 128 * (mi + 1)],
                                    xT[:, SEQ * j + 512 * n:SEQ * j + 512 * (n + 1)],
                                    start=(j == 0),
                                    stop=(j == 7),
                                )
                        for n in range(4):
                            nc.scalar.activation(
                                qk[mi][:, 512 * n:512 * (n + 1)],
                                pq[n][:],
                                AF.Identity,
                                bias=bt[:, mi:mi + 1],
                                scale=1.0,
                            )

                # v projection into [V | 1] layout (ones cols via ACT casts)
                vview = vsb[:].rearrange("p (t h e) -> p t h e", t=16, h=HPC)
                for tt in range(16):
                    nc.scalar.copy(out=vview[:, tt, :, 64:65], in_=ones4[:].unsqueeze(-1))
                    pv = pspr.tile([128, DQ], F32, tag="pproj", name=f"pv{tt}")
                    for j in range(8):
                        nc.tensor.matmul(
                            pv[:],
                            xT[:, SEQ * j + 128 * tt:SEQ * j + 128 * (tt + 1)],
                            wvT[:, 256 * j:256 * (j + 1)],
                            start=(j == 0),
                            stop=False,
                        )
                    nc.tensor.matmul(pv[:], ones1[:], bv_row[:], start=False, stop=True)
                    nc.scalar.copy(
                        out=vview[:, tt, :, 0:64],
                        in_=pv[:].rearrange("p (h d) -> p h d", h=HPC),
                    )

            # ---------- phase 2: attention ----------
            with (
                tc.tile_pool(name="strips", bufs=3) as sp,
                tc.tile_pool(name="ostrips", bufs=2) as op2,
                tc.tile_pool(name="rp", bufs=1) as rp,
                tc.tile_pool(name="bp", bufs=2) as bp,
                tc.tile_pool(name="pss", bufs=2, space="PSUM") as pss,
                tc.tile_pool(name="psc", bufs=1, space="PSUM") as psc,
                tc.tile_pool(name="psb", bufs=1, space="PSUM") as psb,
            ):
                for h in range(HPC):
                    mi, po = h // 2, 64 * (h % 2)
                    att_h = ATT.ap()[h].rearrange("(t p) s -> p t s", p=128)
                    for ch in range(2):
                        sq0 = 1024 * ch
                        halves = [sp.tile([128, 8 * 1024], MMDT, tag="strip",
                                          name=f"strip{h}_{ch}_{i}") for i in range(2)]
                        pc = psc.tile([65, 1024], F32, tag="ctx", name=f"pc{h}_{ch}")
                        for tt in range(16):
                            hs, tl = tt // 8, tt % 8
                            ps = pss.tile([128, 1024], F32, tag="score", name=f"ps{h}_{ch}_{tt}")
                            for n in range(2):
                                nc.tensor.matmul(
                                    ps[:, 512 * n:512 * (n + 1)],
                                    kT[mi][po:po + 64, 128 * tt:128 * (tt + 1)],
                                    qT[mi][po:po + 64, sq0 + 512 * n:sq0 + 512 * (n + 1)],
                                    start=True,
                                    stop=True,
                                )
                            nc.scalar.activation(
                                halves[hs][:, 1024 * tl:1024 * (tl + 1)],
                                ps[:],
                                AF.Exp,
                                scale=0.125,
                            )
                            for n in range(2):
                                nc.tensor.matmul(
                                    pc[:, 512 * n:512 * (n + 1)],
                                    vview[:, tt, h, :],
                                    halves[hs][:, 1024 * tl + 512 * n:1024 * tl + 512 * (n + 1)],
                                    start=(tt == 0),
                                    stop=(tt == 15),
                                )
                        # --- reciprocal of row-sums, on 128 lanes via PE transposes ---
                        zrow = rp.tile([1, 1024], F32, tag="zrow", name=f"zr{h}_{ch}")
                        nc.scalar.copy(out=zrow[:], in_=pc[64:65, :])
                        pzt = psb.tile([128, 1024], F32, tag="bcast", name=f"pzt{h}_{ch}")
                        for g in range(8):
                            nc.tensor.transpose(
                                pzt[:, g:g + 1], zrow[0:1, 128 * g:128 * (g + 1)],
                                ones1[0:1, 0:1],
                            )
                        zt = rp.tile([128, 8], F32, tag="zt", name=f"zt{h}_{ch}")
                        nc.scalar.copy(out=zt[:], in_=pzt[:, 0:8])
                        rt = rp.tile([128, 8], F32, tag="rt", name=f"rt{h}_{ch}")
                        nc.vector.reciprocal(rt[:], zt[:])
                        prr = psb.tile([128, 1024], F32, tag="bcast", name=f"prr{h}_{ch}")
                        for g in range(8):
                            nc.tensor.transpose(
                                prr[0:1, 128 * g:128 * (g + 1)], rt[:, g:g + 1], ident[:]
                            )
                        rrow = rp.tile([1, 1024], F32, tag="rrow", name=f"rr{h}_{ch}")
                        nc.scalar.copy(out=rrow[:], in_=prr[0:1, :])
                        # broadcast recip row to all partitions (PE outer product)
                        pb = psb.tile([128, 1024], F32, tag="bcast", name=f"pb{h}_{ch}")
                        for n in range(2):
                            nc.tensor.matmul(
                                pb[:, 512 * n:512 * (n + 1)],
                                ones1[:],
                                rrow[0:1, 512 * n:512 * (n + 1)],
                                start=True,
                                stop=True,
                            )
                        bcast = bp.tile([128, 1024], F32, tag="bcast_sb", name=f"bc{h}_{ch}")
                        nc.scalar.copy(out=bcast[:], in_=pb[:])
                        # ctx normalize -> ctxT (via SBUF copy; DVE needs <=1 PSUM input)
                        ctmp = rp.tile([64, 1024], F32, tag="ctmp", name=f"ctmp{h}_{ch}")
                        nc.scalar.copy(out=ctmp[:], in_=pc[0:64, :])
                        nc.vector.tensor_mul(
                            ctxT[mi][po:po + 64, sq0:sq0 + 1024], ctmp[:], bcast[0:64, :]
                        )
                        # normalize A^T into f32 out-strips + DMA (all-SBUF muls)
                        for e in range(8):
                            hs, t0 = e // 4, (e % 4) * 2
                            ostrip = op2.tile([128, 2048], F32, tag="ostrip",
                                              name=f"ostrip{h}_{ch}_{e}")
                            for u in range(2):
                                nc.vector.tensor_mul(
                                    ostrip[:, 1024 * u:1024 * (u + 1)],
                                    halves[hs][:, 1024 * (t0 + u):1024 * (t0 + u + 1)].bitcast(F32),
                                    bcast[:],
                                )
                            nc.sync.dma_start(
                                out=att_h[:, 8 * hs + t0:8 * hs + t0 + 2, sq0:sq0 + 1024],
                                in_=ostrip[:].rearrange("p (t s) -> p t s", t=2),
                            )

            # ---------- phase 3: output projection (partial) ----------
            with (
                tc.tile_pool(name="osb", bufs=2) as op,
                tc.tile_pool(name="pso", bufs=4, space="PSUM") as pso,
            ):
                for dt_ in range(8):
                    osb = op.tile([128, SEQ], F32, tag="outT", name=f"osb{dt_}")
                    po = [pso.tile([128, 512], F32, tag="po", name=f"po{dt_}_{n}")
                          for n in range(4)]
                    for it in range(2):
                        for n in range(4):
                            nc.tensor.matmul(
                                po[n][:],
                                woT[it][:, 128 * dt_:128 * (dt_ + 1)],
                                ctxT[it][:, 512 * n:512 * (n + 1)],
                                start=(it == 0),
                                stop=(it == 1),
                            )
                    for n in range(4):
                        nc.scalar.copy(out=osb[:, 512 * n:512 * (n + 1)], in_=po[n][:])
                    nc.sync.dma_start(
                        out=OUTT.ap()[128 * dt_:128 * (dt_ + 1), :], in_=osb[:]
                    )

    nc.finalize()
    return nc


def _get_nc():
    if "nc" not in _state:
        _state["nc"] = _build_nc()
    return _state["nc"]


def _shard_inputs(x, Wq, bq, Wk, bk, Wv, bv, Wo, bo):
    f = lambda a: np.ascontiguousarray(np.asarray(a, dtype=np.float32))
    x, Wq, bq, Wk, bk, Wv, bv, Wo, bo = map(f, (x, Wq, bq, Wk, bk, Wv, bv, Wo, bo))
    xT = [np.ascontiguousarray(x[b].T) for b in range(BSZ)]            # [E, S]
    WqT, WkT, WvT = Wq.T, Wk.T, Wv.T                                   # [E, DQall]
    WoT = np.ascontiguousarray(Wo.T)                                   # [DQall, E]
    in_maps = []
    for c in range(NCORES):
        b, hb = c // 4, c % 4
        sl = slice(DQ * hb, DQ * (hb + 1))
        in_maps.append({
            "xt": xT[b],
            "wqt": np.ascontiguousarray(WqT[:, sl]), "bq": np.ascontiguousarray(bq[sl]),
            "wkt": np.ascontiguousarray(WkT[:, sl]), "bk": np.ascontiguousarray(bk[sl]),
            "wvt": np.ascontiguousarray(WvT[:, sl]), "bv": np.ascontiguousarray(bv[sl]),
            "wot": np.ascontiguousarray(WoT[sl, :]),
        })
    return in_maps, bo


def kernel(x, Wq, bq, Wk, bk, Wv, bv, Wo, bo):
    from concourse.bass_utils import run_bass_kernel_spmd

    nc = _get_nc()
    in_maps, bo_np = _shard_inputs(x, Wq, bq, Wk, bk, Wv, bv, Wo, bo)
    res = run_bass_kernel_spmd(nc, in_maps, core_ids=list(range(NCORES)))
    _state["last"] = res

    attnT = np.empty((BSZ, NHEADS, SEQ, SEQ), np.float32)
    out = np.zeros((BSZ, SEQ, EMBED), np.float32)
    for c in range(NCORES):
        b, hb = c // 4, c % 4
        r = res.results[c]
        attnT[b, HPC * hb:HPC * (hb + 1)] = r["attnT"]
        out[b] += r["outT"].T
    out += bo_np
    return out, attnT.swapaxes(2, 3)
